# revision 15
# baseline (speedup 1.0000x reference)
"""Two-layer LSTM (linear cell/output activations) + FC head on 8 NeuronCores.

Strategy (data-parallel over batch, per the sharding hint):
  - B=32 split across 8 cores -> B_local=4 per core; weights replicated (bf16).
  - Transposed state: h^T/c^T are [H on partitions, (k,b) on free]; per-step
    recurrence is z^T += U^T @ h^T with U tiles stationary.
  - z lives in PSUM: the per-chunk input projection (x@W0 / h0@W1) matmuls
    write xw straight into PSUM, and the per-step recurrence matmuls
    accumulate on top (start=False) -- no separate xw+z vector add.
  - The two layers are software-pipelined with a one-chunk skew: layer-1
    steps of chunk c interleave with layer-0 steps of chunk c+1, so each
    engine (PE / ACT / DVE) always has an independent second stream to hide
    the other layer's cross-engine dependency latency.
  - T=2048 in 64 chunks of 32 steps; steady state runs in a For_i hardware
    loop whose body covers two chunk-pairs (even/odd buffer parity).
  - Gate math per step: one sigmoid ACT op (i,f,o), then 4 DVE ops
    (ig=i*g, c=f*c, c+=ig, h=o*c).  Gate order in SBUF/PSUM is (i,f,o,g)
    via a host-side column permutation.
"""

import os
import numpy as np
from contextlib import ExitStack

os.environ.setdefault("MYCRO_LOCAL_CACHE", "1")

B, T, I, H, O = 32, 2048, 128, 256, 128
NCORES = 8
BL = B // NCORES          # 4 batch elements per core
CHUNK = 32                # timesteps per chunk (PSUM: 2 banks per chunk-z)
NCH = T // CHUNK          # 64 chunks
G4 = 4 * H                # 1024 gate columns
NM = G4 // 128            # 8 gate chunks of 128
KT = H // 128             # 2 contraction tiles

_cache = {}
DIAG_MODE = "full"   # "full" | "mm_only" | "vec_only"  (diag builds only)


def _build(tiny=False, nrep=1, with_bias=False, sim_nch=None):
    import concourse.bacc as bacc
    import concourse.bass as bass
    import concourse.tile as tile
    import concourse.mybir as mybir

    f32 = mybir.dt.float32
    mdt = mybir.dt.bfloat16
    AF = mybir.ActivationFunctionType
    ALU = mybir.AluOpType

    nc = bacc.Bacc("TRN2", target_bir_lowering=False, debug=False,
                   num_devices=NCORES)

    xprep_d = nc.declare_dram_parameter("xprep", [I, BL, T], mdt, isOutput=False)
    w0_d = nc.declare_dram_parameter("w0", [I, G4], mdt, isOutput=False)
    u0_d = nc.declare_dram_parameter("u0", [H, G4], mdt, isOutput=False)
    w1_d = nc.declare_dram_parameter("w1", [H, G4], mdt, isOutput=False)
    u1_d = nc.declare_dram_parameter("u1", [H, G4], mdt, isOutput=False)
    wfc_d = nc.declare_dram_parameter("wfc", [H, O], mdt, isOutput=False)
    bfct_d = nc.declare_dram_parameter("bfct", [128, 1], f32, isOutput=False)
    if with_bias:
        b0r_d = nc.declare_dram_parameter("b0r", [1, G4], mdt, isOutput=False)
        b1r_d = nc.declare_dram_parameter("b1r", [1, G4], mdt, isOutput=False)
    out_d = nc.declare_dram_parameter("outT", [O, BL], f32, isOutput=True)

    with tile.TileContext(nc) as tc, ExitStack() as ctx:
        if tiny:
            pool = ctx.enter_context(tc.tile_pool(name="tp", bufs=1))
            t1 = pool.tile([128, BL], mdt, tag="t1")
            t2 = pool.tile([128, BL], f32, tag="t2")
            nc.sync.dma_start(t1[:, :], xprep_d[:, :, 0])
            nc.vector.tensor_copy(t2[:, :], t1[:, :])
            nc.sync.dma_start(out_d[:, :], t2[:, :])
            nc.compile()
            return nc

        const = ctx.enter_context(tc.tile_pool(name="const", bufs=1))
        work = ctx.enter_context(tc.tile_pool(name="work", bufs=3))
        psum = ctx.enter_context(tc.tile_pool(name="psum", bufs=1, space="PSUM"))

        # ---- persistent SBUF residents ----
        xall = const.tile([128, BL * T], mdt, tag="xall")       # col = b*T + t
        w0 = const.tile([128, G4], mdt, tag="w0")
        u0 = [const.tile([128, G4], mdt, tag=f"u0_{k}", name=f"u0_{k}")
              for k in range(KT)]
        w1 = [const.tile([128, G4], mdt, tag=f"w1_{k}", name=f"w1_{k}")
              for k in range(KT)]
        u1 = [const.tile([128, G4], mdt, tag=f"u1_{k}", name=f"u1_{k}")
              for k in range(KT)]
        wf = [const.tile([128, O], mdt, tag=f"wf_{k}", name=f"wf_{k}")
              for k in range(KT)]
        bfct = const.tile([128, 1], f32, tag="bfct")
        if with_bias:
            b0r = const.tile([1, G4], mdt, tag="b0r")
            b1r = const.tile([1, G4], mdt, tag="b1r")
            ones = const.tile([1, BL * CHUNK], mdt, tag="ones")
        # h0 stream buffers (double-buffered): col = k*128 + b*32 + t
        h0t = [const.tile([128, KT * BL * CHUNK], mdt, tag=f"h0t_{p}",
                          name=f"h0t_{p}") for p in range(2)]
        # recurrent state, col = k*BL + b
        c0 = const.tile([128, KT * BL], f32, tag="c0")
        c1 = const.tile([128, KT * BL], f32, tag="c1")
        h1 = const.tile([128, KT * BL], mdt, tag="h1")

        # ---- PSUM: 4 tiles x 2 banks = 8 banks ----
        zx0 = [psum.tile([128, NM * BL * CHUNK], f32, tag=f"zx0_{p}",
                         name=f"zx0_{p}") for p in range(2)]
        zx1 = [psum.tile([128, NM * BL * CHUNK], f32, tag=f"zx1_{p}",
                         name=f"zx1_{p}") for p in range(2)]

        nc.sync.dma_start(xall[:, :].rearrange("p (b t) -> p b t", b=BL),
                          xprep_d[:, :, :])
        nc.sync.dma_start(w0[:, :], w0_d[:, :])
        for k in range(KT):
            sl = slice(k * 128, (k + 1) * 128)
            nc.sync.dma_start(u0[k][:, :], u0_d[sl, :])
            nc.sync.dma_start(w1[k][:, :], w1_d[sl, :])
            nc.sync.dma_start(u1[k][:, :], u1_d[sl, :])
            nc.sync.dma_start(wf[k][:, :], wfc_d[sl, :])
        nc.sync.dma_start(bfct[:, :], bfct_d[:, :])
        if with_bias:
            nc.sync.dma_start(b0r[:, :], b0r_d[:, :])
            nc.sync.dma_start(b1r[:, :], b1r_d[:, :])
            nc.vector.memset(ones[:, :], 1.0)

        for p in range(2):
            nc.vector.memset(h0t[p][:, :], 0.0)
        nc.vector.memset(c0[:, :], 0.0)
        nc.vector.memset(c1[:, :], 0.0)
        nc.vector.memset(h1[:, :], 0.0)

        xall_r = xall[:, :].rearrange("p (b t) -> p b t", b=BL)

        def zx_r(z):
            return z[:, :].rearrange("p (m b t) -> p m b t", m=NM, b=BL)

        def h0t_r(p):
            return h0t[p][:, :].rearrange("p (k b t) -> p k b t", k=KT, b=BL)

        def proj0(zx, toff):
            """xw0 = x[:, :, toff:toff+CHUNK] @ W0 -> PSUM zx."""
            if isinstance(toff, int):
                rhs = xall_r[:, :, toff:toff + CHUNK]
            else:
                rhs = xall_r[:, :, bass.ds(toff, CHUNK)]
            for m in range(NM):
                msl = slice(m * 128, (m + 1) * 128)
                nc.tensor.matmul(zx[:, msl], lhsT=w0[:, msl], rhs=rhs,
                                 start=(m % 4 == 0), stop=False)
            if with_bias:
                for m in range(NM):
                    msl = slice(m * 128, (m + 1) * 128)
                    nc.tensor.matmul(zx[:, msl], lhsT=b0r[:, msl],
                                     rhs=ones[:, :], start=False, stop=False)

        def proj1(zx, hp):
            """xw1 = h0t[hp] @ W1 -> PSUM zx."""
            for m in range(NM):
                msl = slice(m * 128, (m + 1) * 128)
                for k in range(KT):
                    nc.tensor.matmul(
                        zx[:, msl], lhsT=w1[k][:, msl],
                        rhs=h0t[hp][:, k * BL * CHUNK:(k + 1) * BL * CHUNK],
                        start=(m % 4 == 0 and k == 0), stop=False)
            if with_bias:
                for m in range(NM):
                    msl = slice(m * 128, (m + 1) * 128)
                    nc.tensor.matmul(zx[:, msl], lhsT=b1r[:, msl],
                                     rhs=ones[:, :], start=False, stop=False)

        def rec_mms(zxr, t, u, h_rhs):
            """z[:, :, t] += U^T @ h for one step.

            stop=True only on the last matmul per PSUM bank per chunk epoch
            (the group-started flag is bank-granular)."""
            for m in range(NM):
                msl = slice(m * 128, (m + 1) * 128)
                for k in range(KT):
                    stop = (t == CHUNK - 1 and k == KT - 1
                            and m in (NM // 2 - 1, NM - 1))
                    nc.tensor.matmul(zxr[:, m, :, t], lhsT=u[k][:, msl],
                                     rhs=h_rhs(k),
                                     start=False, stop=stop)

        def gate_math(zxr, t, cst, h_out_ap, lname, eng=None):
            """sigmoid + cell update + h for one step; z read from PSUM.

            eng selects the engine for the SBUF-only ops (fc/add/h) so the
            two layers' chains can run on different engines (DVE vs GpSimd);
            the g-gate mult (PSUM operand) always runs on DVE."""
            if eng is None:
                eng = nc.vector
            sig = work.tile([128, 6 * BL], f32, tag=f"sig{lname}")
            sig_r = sig[:, :].rearrange("p (m b) -> p m b", m=6)
            nc.scalar.activation(sig_r, zxr[:, 0:6, :, t], AF.Sigmoid)
            ig = work.tile([128, KT * BL], f32, tag=f"ig{lname}")
            ig_r = ig[:, :].rearrange("p (k b) -> p k b", k=KT)
            cst_r = cst[:, :].rearrange("p (k b) -> p k b", k=KT)
            nc.vector.tensor_tensor(ig_r, sig_r[:, 0:2, :],
                                    zxr[:, 6:8, :, t], ALU.mult)
            eng.tensor_tensor(cst_r, sig_r[:, 2:4, :], cst_r, ALU.mult)
            eng.tensor_tensor(cst_r, cst_r, ig_r, ALU.add)
            eng.tensor_tensor(h_out_ap, sig_r[:, 4:6, :], cst_r,
                              ALU.mult)

        def l0_step(t, zp, hp, sig_o_emit=None):
            """Layer-0 step t of the chunk in h0t[hp] / zx0[zp]."""
            zr = zx_r(zx0[zp])
            hr = h0t_r(hp)
            hprev = h0t_r(1 - hp)

            def h_rhs(k):
                if t == 0:
                    return hprev[:, k, :, CHUNK - 1]
                return hr[:, k, :, t - 1]

            if DIAG_MODE != "vec_only":
                rec_mms(zr, t, u0, h_rhs)
            if DIAG_MODE != "mm_only":
                gate_math(zr, t, c0, hr[:, :, :, t], "0")

        def l1_step(t, zp, sig_o_emit=None):
            zr = zx_r(zx1[zp])
            h1_r = h1[:, :].rearrange("p (k b) -> p k b", k=KT)

            def h_rhs(k):
                return h1_r[:, k, :]

            if DIAG_MODE != "vec_only":
                rec_mms(zr, t, u1, h_rhs)
            if DIAG_MODE != "mm_only":
                gate_math(zr, t, c1, h1_r, "1")

        def half(l1_par, l0_par, l0_off, skip_l0=False, skip_l1=False):
            """One pipeline half: L0 on chunk with buffers l0_par at x-offset
            l0_off, interleaved with L1 on the chunk held in parity l1_par."""
            if not skip_l0:
                proj0(zx0[l0_par], l0_off)
            if not skip_l1:
                proj1(zx1[l1_par], l1_par)
            for t in range(CHUNK):
                sig_o = []
                if not skip_l0:
                    l0_step(t, l0_par, l0_par, sig_o)
                if not skip_l1:
                    l1_step(t, l1_par, sig_o)
                for emit in sig_o:
                    emit()

        # ---- prologue: L0 chunk 0 (parity 0) ----
        half(0, 0, 0, skip_l1=True)

        nch = NCH if sim_nch is None else sim_nch
        if sim_nch is not None:
            # fully unrolled small-T variant for TimelineSim (no registers)
            for i in range(nch // 2 - 1):
                half(0, 1, (2 * i + 1) * CHUNK)
                half(1, 0, (2 * i + 2) * CHUNK)
        else:
            # steady state: body i does L1(2i),L0(2i+1) ; L1(2i+1),L0(2i+2)
            rep_ctx = tc.For_i(0, nrep) if nrep > 1 else None
            if rep_ctx is not None:
                rep_ctx.__enter__()
            with tc.For_i(0, (NCH // 2 - 1) * 2 * CHUNK, 2 * CHUNK) as iv:
                half(0, 1, iv + CHUNK)
                half(1, 0, iv + 2 * CHUNK)
            if rep_ctx is not None:
                rep_ctx.__exit__(None, None, None)

        # ---- epilogue: L1(nch-2) + L0(nch-1); then L1(nch-1) ----
        half(0, 1, (nch - 1) * CHUNK)
        half(1, 0, 0, skip_l0=True)

        # ---- FC head: out^T = Wfc^T @ h1^T + bfc ----
        psf = zx1[0][:, 0:BL]
        for k in range(KT):
            nc.tensor.matmul(psf, lhsT=wf[k][:, :],
                             rhs=h1[:, k * BL:(k + 1) * BL],
                             start=(k == 0), stop=(k == KT - 1))
        oT = work.tile([128, BL], f32, tag="oT")
        nc.scalar.activation(oT[:, :], psf, AF.Identity, bias=bfct[:, 0:1])
        nc.sync.dma_start(out_d[:, :], oT[:, :])

    nc.compile()
    return nc


def _get_compiled(with_bias=False):
    key = ("bias" if with_bias else "main")
    if key not in _cache:
        _cache[key] = _build(with_bias=with_bias)
    return _cache[key]


def _in_maps(input_seq, W0, U0, b0, W1, U1, b1, Wfc, bfc, with_bias=False):
    import ml_dtypes
    mdt = ml_dtypes.bfloat16
    x = np.asarray(input_seq, dtype=np.float32)
    # reorder gate blocks (i,f,g,o) -> (i,f,o,g)
    perm = np.concatenate([np.arange(0, 2 * H),
                           np.arange(3 * H, 4 * H),
                           np.arange(2 * H, 3 * H)])

    def gp(w):
        return np.ascontiguousarray(
            np.asarray(w, np.float32)[..., perm].astype(mdt))

    shared = {
        "w0": gp(W0),
        "u0": gp(U0),
        "w1": gp(W1),
        "u1": gp(U1),
        "wfc": np.ascontiguousarray(np.asarray(Wfc, np.float32).astype(mdt)),
        "bfct": np.ascontiguousarray(
            np.asarray(bfc, np.float32).reshape(1, 128).T),
    }
    if with_bias:
        shared["b0r"] = gp(np.asarray(b0, np.float32).reshape(1, G4))
        shared["b1r"] = gp(np.asarray(b1, np.float32).reshape(1, G4))
    in_maps = []
    for c in range(NCORES):
        xs = x[c * BL:(c + 1) * BL]                       # [BL, T, I]
        xp = np.ascontiguousarray(xs.transpose(2, 0, 1).astype(mdt))
        m = dict(shared)
        m["xprep"] = xp
        in_maps.append(m)
    return in_maps


def _run(nc, inputs, with_bias=False):
    from concourse.bass_utils import run_bass_kernel_spmd
    in_maps = _in_maps(**inputs, with_bias=with_bias)
    res = run_bass_kernel_spmd(nc, in_maps, list(range(NCORES)))
    out = np.empty((B, 1, O), np.float32)
    for c in range(NCORES):
        out[c * BL:(c + 1) * BL, 0, :] = res.results[c]["outT"].T
    return out


def kernel(input_seq, W0, U0, b0, W1, U1, b1, Wfc, bfc):
    with_bias = bool(np.any(np.asarray(b0)) or np.any(np.asarray(b1)))
    nc = _get_compiled(with_bias=with_bias)
    return _run(nc, dict(input_seq=input_seq, W0=W0, U0=U0, b0=b0, W1=W1,
                         U1=U1, b1=b1, Wfc=Wfc, bfc=bfc), with_bias=with_bias)


# revision 18
# speedup vs baseline: 1.1616x; 1.1616x over previous
"""Two-layer LSTM (linear cell/output activations) + FC head on 8 NeuronCores.

Strategy (data-parallel over batch, per the sharding hint):
  - B=32 split across 8 cores -> B_local=4 per core; weights replicated (bf16).
  - Transposed state: h^T/c^T are [H on partitions, (k,b) on free]; per-step
    recurrence is z^T += U^T @ h^T with U tiles stationary.
  - z lives in PSUM: the per-chunk input projection (x@W0 / h0@W1) matmuls
    write xw straight into PSUM, and the per-step recurrence matmuls
    accumulate on top (start=False) -- no separate xw+z vector add.
  - The two layers are software-pipelined with a one-chunk skew: layer-1
    steps of chunk c interleave with layer-0 steps of chunk c+1, so each
    engine (PE / ACT / DVE) always has an independent second stream to hide
    the other layer's cross-engine dependency latency.
  - T=2048 in 64 chunks of 32 steps; steady state runs in a For_i hardware
    loop whose body covers two chunk-pairs (even/odd buffer parity).
  - Gate math per step: one sigmoid ACT op (i,f,o), then 4 DVE ops
    (ig=i*g, c=f*c, c+=ig, h=o*c).  Gate order in SBUF/PSUM is (i,f,o,g)
    via a host-side column permutation.
"""

import os
import numpy as np
from contextlib import ExitStack

os.environ.setdefault("MYCRO_LOCAL_CACHE", "1")

B, T, I, H, O = 32, 2048, 128, 256, 128
NCORES = 8
BL = B // NCORES          # 4 batch elements per core
CHUNK = 32                # timesteps per chunk (PSUM: 2 banks per chunk-z)
NCH = T // CHUNK          # 64 chunks
G4 = 4 * H                # 1024 gate columns
NM = G4 // 128            # 8 gate chunks of 128
KT = H // 128             # 2 contraction tiles

_cache = {}
DIAG_MODE = "full"   # "full" | "mm_only" | "vec_only"  (diag builds only)


def _build(tiny=False, nrep=1, with_bias=False, sim_nch=None):
    import concourse.bacc as bacc
    import concourse.bass as bass
    import concourse.tile as tile
    import concourse.mybir as mybir

    f32 = mybir.dt.float32
    mdt = mybir.dt.bfloat16
    AF = mybir.ActivationFunctionType
    ALU = mybir.AluOpType

    nc = bacc.Bacc("TRN2", target_bir_lowering=False, debug=False,
                   num_devices=NCORES)

    xprep_d = nc.declare_dram_parameter("xprep", [I, BL, T], mdt, isOutput=False)
    w0_d = nc.declare_dram_parameter("w0", [I, G4], mdt, isOutput=False)
    u0_d = nc.declare_dram_parameter("u0", [H, G4], mdt, isOutput=False)
    w1_d = nc.declare_dram_parameter("w1", [H, G4], mdt, isOutput=False)
    u1_d = nc.declare_dram_parameter("u1", [H, G4], mdt, isOutput=False)
    wfc_d = nc.declare_dram_parameter("wfc", [H, O], mdt, isOutput=False)
    bfct_d = nc.declare_dram_parameter("bfct", [128, 1], f32, isOutput=False)
    if with_bias:
        b0r_d = nc.declare_dram_parameter("b0r", [1, G4], mdt, isOutput=False)
        b1r_d = nc.declare_dram_parameter("b1r", [1, G4], mdt, isOutput=False)
    out_d = nc.declare_dram_parameter("outT", [O, BL], f32, isOutput=True)

    with tile.TileContext(nc) as tc, ExitStack() as ctx:
        if tiny:
            pool = ctx.enter_context(tc.tile_pool(name="tp", bufs=1))
            t1 = pool.tile([128, BL], mdt, tag="t1")
            t2 = pool.tile([128, BL], f32, tag="t2")
            nc.sync.dma_start(t1[:, :], xprep_d[:, :, 0])
            nc.vector.tensor_copy(t2[:, :], t1[:, :])
            nc.sync.dma_start(out_d[:, :], t2[:, :])
            nc.compile()
            return nc

        const = ctx.enter_context(tc.tile_pool(name="const", bufs=1))
        work = ctx.enter_context(tc.tile_pool(name="work", bufs=3))
        psum = ctx.enter_context(tc.tile_pool(name="psum", bufs=1, space="PSUM"))

        # ---- persistent SBUF residents ----
        xall = const.tile([128, BL * T], mdt, tag="xall")       # col = b*T + t
        w0 = const.tile([128, G4], mdt, tag="w0")
        u0 = [const.tile([128, G4], mdt, tag=f"u0_{k}", name=f"u0_{k}")
              for k in range(KT)]
        w1 = [const.tile([128, G4], mdt, tag=f"w1_{k}", name=f"w1_{k}")
              for k in range(KT)]
        u1 = [const.tile([128, G4], mdt, tag=f"u1_{k}", name=f"u1_{k}")
              for k in range(KT)]
        wf = [const.tile([128, O], mdt, tag=f"wf_{k}", name=f"wf_{k}")
              for k in range(KT)]
        bfct = const.tile([128, 1], f32, tag="bfct")
        if with_bias:
            b0r = const.tile([1, G4], mdt, tag="b0r")
            b1r = const.tile([1, G4], mdt, tag="b1r")
            ones = const.tile([1, BL * CHUNK], mdt, tag="ones")
        # h0 stream buffers (double-buffered): col = k*128 + b*32 + t
        h0t = [const.tile([128, KT * BL * CHUNK], mdt, tag=f"h0t_{p}",
                          name=f"h0t_{p}") for p in range(2)]
        # recurrent state, col = k*BL + b
        c0 = const.tile([128, KT * BL], f32, tag="c0")
        c1 = const.tile([128, KT * BL], f32, tag="c1")
        h1 = const.tile([128, KT * BL], mdt, tag="h1")

        # ---- PSUM: 4 tiles x 2 banks = 8 banks ----
        zx0 = [psum.tile([128, NM * BL * CHUNK], f32, tag=f"zx0_{p}",
                         name=f"zx0_{p}") for p in range(2)]
        zx1 = [psum.tile([128, NM * BL * CHUNK], f32, tag=f"zx1_{p}",
                         name=f"zx1_{p}") for p in range(2)]

        nc.sync.dma_start(xall[:, :].rearrange("p (b t) -> p b t", b=BL),
                          xprep_d[:, :, :])
        nc.sync.dma_start(w0[:, :], w0_d[:, :])
        for k in range(KT):
            sl = slice(k * 128, (k + 1) * 128)
            nc.sync.dma_start(u0[k][:, :], u0_d[sl, :])
            nc.sync.dma_start(w1[k][:, :], w1_d[sl, :])
            nc.sync.dma_start(u1[k][:, :], u1_d[sl, :])
            nc.sync.dma_start(wf[k][:, :], wfc_d[sl, :])
        nc.sync.dma_start(bfct[:, :], bfct_d[:, :])
        if with_bias:
            nc.sync.dma_start(b0r[:, :], b0r_d[:, :])
            nc.sync.dma_start(b1r[:, :], b1r_d[:, :])
            nc.vector.memset(ones[:, :], 1.0)

        for p in range(2):
            nc.vector.memset(h0t[p][:, :], 0.0)
        nc.vector.memset(c0[:, :], 0.0)
        nc.vector.memset(c1[:, :], 0.0)
        nc.vector.memset(h1[:, :], 0.0)

        xall_r = xall[:, :].rearrange("p (b t) -> p b t", b=BL)

        def zx_r(z):
            return z[:, :].rearrange("p (m b t) -> p m b t", m=NM, b=BL)

        def h0t_r(p):
            return h0t[p][:, :].rearrange("p (k b t) -> p k b t", k=KT, b=BL)

        def proj0(zx, toff):
            """xw0 = x[:, :, toff:toff+CHUNK] @ W0 -> PSUM zx."""
            if isinstance(toff, int):
                rhs = xall_r[:, :, toff:toff + CHUNK]
            else:
                rhs = xall_r[:, :, bass.ds(toff, CHUNK)]
            for m in range(NM):
                msl = slice(m * 128, (m + 1) * 128)
                nc.tensor.matmul(zx[:, msl], lhsT=w0[:, msl], rhs=rhs,
                                 start=(m % 4 == 0), stop=False)
            if with_bias:
                for m in range(NM):
                    msl = slice(m * 128, (m + 1) * 128)
                    nc.tensor.matmul(zx[:, msl], lhsT=b0r[:, msl],
                                     rhs=ones[:, :], start=False, stop=False)

        def proj1(zx, hp):
            """xw1 = h0t[hp] @ W1 -> PSUM zx."""
            for m in range(NM):
                msl = slice(m * 128, (m + 1) * 128)
                for k in range(KT):
                    nc.tensor.matmul(
                        zx[:, msl], lhsT=w1[k][:, msl],
                        rhs=h0t[hp][:, k * BL * CHUNK:(k + 1) * BL * CHUNK],
                        start=(m % 4 == 0 and k == 0), stop=False)
            if with_bias:
                for m in range(NM):
                    msl = slice(m * 128, (m + 1) * 128)
                    nc.tensor.matmul(zx[:, msl], lhsT=b1r[:, msl],
                                     rhs=ones[:, :], start=False, stop=False)

        def rec_mms(zxr, t, u, h_rhs):
            """z[:, :, t] += U^T @ h for one step.

            stop=True only on the last matmul per PSUM bank per chunk epoch
            (the group-started flag is bank-granular)."""
            for m in range(NM):
                msl = slice(m * 128, (m + 1) * 128)
                for k in range(KT):
                    stop = (t == CHUNK - 1 and k == KT - 1
                            and m in (NM // 2 - 1, NM - 1))
                    nc.tensor.matmul(zxr[:, m, :, t], lhsT=u[k][:, msl],
                                     rhs=h_rhs(k),
                                     start=False, stop=stop)

        def gate_math(zxr, t, cst, h_out_ap, lname, eng=None):
            """sigmoid + cell update + h for one step; z read from PSUM.

            eng selects the engine for the SBUF-only ops (fc/add/h) so the
            two layers' chains can run on different engines (DVE vs GpSimd);
            the g-gate mult (PSUM operand) always runs on DVE."""
            if eng is None:
                eng = nc.vector
            sig = work.tile([128, 6 * BL], f32, tag=f"sig{lname}")
            sig_r = sig[:, :].rearrange("p (m b) -> p m b", m=6)
            nc.scalar.activation(sig_r, zxr[:, 0:6, :, t], AF.Sigmoid)
            ig = work.tile([128, KT * BL], f32, tag=f"ig{lname}")
            ig_r = ig[:, :].rearrange("p (k b) -> p k b", k=KT)
            cst_r = cst[:, :].rearrange("p (k b) -> p k b", k=KT)
            nc.vector.tensor_tensor(ig_r, sig_r[:, 0:2, :],
                                    zxr[:, 6:8, :, t], ALU.mult)
            eng.tensor_tensor(cst_r, sig_r[:, 2:4, :], cst_r, ALU.mult)
            eng.tensor_tensor(cst_r, cst_r, ig_r, ALU.add)
            eng.tensor_tensor(h_out_ap, sig_r[:, 4:6, :], cst_r,
                              ALU.mult)

        def l0_step(t, zp, hp):
            """Layer-0 step t of the chunk in h0t[hp] / zx0[zp]."""
            zr = zx_r(zx0[zp])
            hr = h0t_r(hp)
            hprev = h0t_r(1 - hp)

            def h_rhs(k):
                if t == 0:
                    return hprev[:, k, :, CHUNK - 1]
                return hr[:, k, :, t - 1]

            if DIAG_MODE != "vec_only":
                rec_mms(zr, t, u0, h_rhs)
            if DIAG_MODE != "mm_only":
                gate_math(zr, t, c0, hr[:, :, :, t], "0")

        def l1_step(t, zp):
            zr = zx_r(zx1[zp])
            h1_r = h1[:, :].rearrange("p (k b) -> p k b", k=KT)

            def h_rhs(k):
                return h1_r[:, k, :]

            if DIAG_MODE != "vec_only":
                rec_mms(zr, t, u1, h_rhs)
            if DIAG_MODE != "mm_only":
                gate_math(zr, t, c1, h1_r, "1")

        def half(l1_par, l0_par, l0_off, skip_l0=False, skip_l1=False):
            """One pipeline half: L0 on chunk with buffers l0_par at x-offset
            l0_off, interleaved with L1 on the chunk held in parity l1_par."""
            if not skip_l0:
                proj0(zx0[l0_par], l0_off)
            if not skip_l1:
                proj1(zx1[l1_par], l1_par)
            for t in range(CHUNK):
                if not skip_l0:
                    l0_step(t, l0_par, l0_par)
                if not skip_l1:
                    l1_step(t, l1_par)

        # ---- prologue: L0 chunk 0 (parity 0) ----
        half(0, 0, 0, skip_l1=True)

        nch = NCH if sim_nch is None else sim_nch
        if sim_nch is not None:
            # fully unrolled small-T variant for TimelineSim (no registers)
            for i in range(nch // 2 - 1):
                half(0, 1, (2 * i + 1) * CHUNK)
                half(1, 0, (2 * i + 2) * CHUNK)
        else:
            # steady state: body i does L1(2i),L0(2i+1) ; L1(2i+1),L0(2i+2)
            rep_ctx = tc.For_i(0, nrep) if nrep > 1 else None
            if rep_ctx is not None:
                rep_ctx.__enter__()
            with tc.For_i(0, (NCH // 2 - 1) * 2 * CHUNK, 2 * CHUNK) as iv:
                half(0, 1, iv + CHUNK)
                half(1, 0, iv + 2 * CHUNK)
            if rep_ctx is not None:
                rep_ctx.__exit__(None, None, None)

        # ---- epilogue: L1(nch-2) + L0(nch-1); then L1(nch-1) ----
        half(0, 1, (nch - 1) * CHUNK)
        half(1, 0, 0, skip_l0=True)

        # ---- FC head: out^T = Wfc^T @ h1^T + bfc ----
        psf = zx1[0][:, 0:BL]
        for k in range(KT):
            nc.tensor.matmul(psf, lhsT=wf[k][:, :],
                             rhs=h1[:, k * BL:(k + 1) * BL],
                             start=(k == 0), stop=(k == KT - 1))
        oT = work.tile([128, BL], f32, tag="oT")
        nc.scalar.activation(oT[:, :], psf, AF.Identity, bias=bfct[:, 0:1])
        nc.sync.dma_start(out_d[:, :], oT[:, :])

    nc.compile()
    return nc


def _get_compiled(with_bias=False):
    key = ("bias" if with_bias else "main")
    if key not in _cache:
        _cache[key] = _build(with_bias=with_bias)
    return _cache[key]


def _in_maps(input_seq, W0, U0, b0, W1, U1, b1, Wfc, bfc, with_bias=False):
    import ml_dtypes
    mdt = ml_dtypes.bfloat16
    x = np.asarray(input_seq, dtype=np.float32)
    # reorder gate blocks (i,f,g,o) -> (i,f,o,g)
    perm = np.concatenate([np.arange(0, 2 * H),
                           np.arange(3 * H, 4 * H),
                           np.arange(2 * H, 3 * H)])

    def gp(w):
        return np.ascontiguousarray(
            np.asarray(w, np.float32)[..., perm].astype(mdt))

    shared = {
        "w0": gp(W0),
        "u0": gp(U0),
        "w1": gp(W1),
        "u1": gp(U1),
        "wfc": np.ascontiguousarray(np.asarray(Wfc, np.float32).astype(mdt)),
        "bfct": np.ascontiguousarray(
            np.asarray(bfc, np.float32).reshape(1, 128).T),
    }
    if with_bias:
        shared["b0r"] = gp(np.asarray(b0, np.float32).reshape(1, G4))
        shared["b1r"] = gp(np.asarray(b1, np.float32).reshape(1, G4))
    in_maps = []
    for c in range(NCORES):
        xs = x[c * BL:(c + 1) * BL]                       # [BL, T, I]
        xp = np.ascontiguousarray(xs.transpose(2, 0, 1).astype(mdt))
        m = dict(shared)
        m["xprep"] = xp
        in_maps.append(m)
    return in_maps


def _run(nc, inputs, with_bias=False):
    from concourse.bass_utils import run_bass_kernel_spmd
    in_maps = _in_maps(**inputs, with_bias=with_bias)
    res = run_bass_kernel_spmd(nc, in_maps, list(range(NCORES)))
    out = np.empty((B, 1, O), np.float32)
    for c in range(NCORES):
        out[c * BL:(c + 1) * BL, 0, :] = res.results[c]["outT"].T
    return out


def kernel(input_seq, W0, U0, b0, W1, U1, b1, Wfc, bfc):
    with_bias = bool(np.any(np.asarray(b0)) or np.any(np.asarray(b1)))
    nc = _get_compiled(with_bias=with_bias)
    return _run(nc, dict(input_seq=input_seq, W0=W0, U0=U0, b0=b0, W1=W1,
                         U1=U1, b1=b1, Wfc=Wfc, bfc=bfc), with_bias=with_bias)


# revision 34
# speedup vs baseline: 1.3561x; 1.1674x over previous
"""Two-layer LSTM (linear cell/output activations) + FC head on 8 NeuronCores.

Strategy (data-parallel over batch, per the sharding hint):
  - B=32 split across 8 cores -> B_local=4 per core; weights replicated (bf16).
  - Transposed state: h^T/c^T are [H on partitions, (k,b) on free]; per-step
    recurrence is z^T += U^T @ h^T with U tiles stationary.
  - z lives in PSUM: the per-chunk input projection (x@W0 / h0@W1) matmuls
    write xw straight into PSUM, and the per-step recurrence matmuls
    accumulate on top (start=False) -- no separate xw+z vector add.
  - The two layers are software-pipelined with a one-chunk skew: layer-1
    steps of chunk c interleave with layer-0 steps of chunk c+1, so each
    engine (PE / ACT / DVE) always has an independent second stream to hide
    the other layer's cross-engine dependency latency.
  - T=2048 in 64 chunks of 32 steps; steady state runs in a For_i hardware
    loop whose body covers two chunk-pairs (even/odd buffer parity).
  - Each chunk-z is two single-bank PSUM tiles (i/f gates, o/g gates); the
    i/f sigmoid is emitted between the bank-0 and bank-1 matmuls so it runs
    concurrently with the o/g matmuls (no tile-level conflict).  Then the
    o-sigmoid + 4 DVE ops (c=f*c, ig=i*g, c+=ig, h=o*c) finish the step.
    Gate order in PSUM is (i,f,o,g) via a host-side column permutation.
"""

import os
import numpy as np
from contextlib import ExitStack

os.environ.setdefault("MYCRO_LOCAL_CACHE", "1")

B, T, I, H, O = 32, 2048, 128, 256, 128
NCORES = 8
BL = B // NCORES          # 4 batch elements per core
CHUNK = 32                # timesteps per chunk (PSUM: 2 banks per chunk-z)
NCH = T // CHUNK          # 64 chunks
G4 = 4 * H                # 1024 gate columns
NM = G4 // 128            # 8 gate chunks of 128
KT = H // 128             # 2 contraction tiles

_cache = {}
DIAG_MODE = "full"   # "full" | "mm_only" | "vec_only"  (diag builds only)


def _build(tiny=False, nrep=1, with_bias=False, sim_nch=None):
    import concourse.bacc as bacc
    import concourse.bass as bass
    import concourse.tile as tile
    import concourse.mybir as mybir

    f32 = mybir.dt.float32
    mdt = mybir.dt.bfloat16
    AF = mybir.ActivationFunctionType
    ALU = mybir.AluOpType

    nc = bacc.Bacc("TRN2", target_bir_lowering=False, debug=False,
                   num_devices=NCORES)

    xprep_d = nc.declare_dram_parameter("xprep", [I, BL, T], mdt, isOutput=False)
    w0_d = nc.declare_dram_parameter("w0", [I, G4], mdt, isOutput=False)
    u0_d = nc.declare_dram_parameter("u0", [H, G4], mdt, isOutput=False)
    w1_d = nc.declare_dram_parameter("w1", [H, G4], mdt, isOutput=False)
    u1_d = nc.declare_dram_parameter("u1", [H, G4], mdt, isOutput=False)
    wfc_d = nc.declare_dram_parameter("wfc", [H, O], mdt, isOutput=False)
    bfct_d = nc.declare_dram_parameter("bfct", [128, 1], f32, isOutput=False)
    if with_bias:
        b0r_d = nc.declare_dram_parameter("b0r", [1, G4], mdt, isOutput=False)
        b1r_d = nc.declare_dram_parameter("b1r", [1, G4], mdt, isOutput=False)
    out_d = nc.declare_dram_parameter("outT", [O, BL], f32, isOutput=True)

    with tile.TileContext(nc) as tc, ExitStack() as ctx:
        if tiny:
            pool = ctx.enter_context(tc.tile_pool(name="tp", bufs=1))
            t1 = pool.tile([128, BL], mdt, tag="t1")
            t2 = pool.tile([128, BL], f32, tag="t2")
            nc.sync.dma_start(t1[:, :], xprep_d[:, :, 0])
            nc.vector.tensor_copy(t2[:, :], t1[:, :])
            nc.sync.dma_start(out_d[:, :], t2[:, :])
            nc.compile()
            return nc

        const = ctx.enter_context(tc.tile_pool(name="const", bufs=1))
        work = ctx.enter_context(tc.tile_pool(name="work", bufs=3))
        psum = ctx.enter_context(tc.tile_pool(name="psum", bufs=1, space="PSUM"))

        # ---- persistent SBUF residents ----
        xall = const.tile([128, BL * T], mdt, tag="xall")       # col = b*T + t
        w0 = const.tile([128, G4], mdt, tag="w0")
        u0 = [const.tile([128, G4], mdt, tag=f"u0_{k}", name=f"u0_{k}")
              for k in range(KT)]
        w1 = [const.tile([128, G4], mdt, tag=f"w1_{k}", name=f"w1_{k}")
              for k in range(KT)]
        u1 = [const.tile([128, G4], mdt, tag=f"u1_{k}", name=f"u1_{k}")
              for k in range(KT)]
        wf = [const.tile([128, O], mdt, tag=f"wf_{k}", name=f"wf_{k}")
              for k in range(KT)]
        bfct = const.tile([128, 1], f32, tag="bfct")
        if with_bias:
            b0r = const.tile([1, G4], mdt, tag="b0r")
            b1r = const.tile([1, G4], mdt, tag="b1r")
            ones = const.tile([1, BL * CHUNK], mdt, tag="ones")
        # h0 stream buffers (double-buffered): col = k*128 + b*32 + t
        h0t = [const.tile([128, KT * BL * CHUNK], mdt, tag=f"h0t_{p}",
                          name=f"h0t_{p}") for p in range(2)]
        # recurrent state, col = k*BL + b
        c0 = const.tile([128, KT * BL], f32, tag="c0")
        c1 = const.tile([128, KT * BL], f32, tag="c1")
        h1 = const.tile([128, KT * BL], mdt, tag="h1")

        # ---- PSUM: 8 single-bank tiles (i/f gates and o/g gates split so a
        # mid-burst sigmoid read of the i/f tile has no tile-level conflict
        # with the o/g matmul writes) ----
        HB = NM * BL * CHUNK // 2   # 512 f32 = one bank
        zx0 = [(psum.tile([128, HB], f32, tag=f"zx0i_{p}", name=f"zx0i_{p}"),
                psum.tile([128, HB], f32, tag=f"zx0g_{p}", name=f"zx0g_{p}"))
               for p in range(2)]
        zx1 = [(psum.tile([128, HB], f32, tag=f"zx1i_{p}", name=f"zx1i_{p}"),
                psum.tile([128, HB], f32, tag=f"zx1g_{p}", name=f"zx1g_{p}"))
               for p in range(2)]

        nc.sync.dma_start(xall[:, :].rearrange("p (b t) -> p b t", b=BL),
                          xprep_d[:, :, :])
        nc.sync.dma_start(w0[:, :], w0_d[:, :])
        for k in range(KT):
            sl = slice(k * 128, (k + 1) * 128)
            nc.sync.dma_start(u0[k][:, :], u0_d[sl, :])
            nc.sync.dma_start(w1[k][:, :], w1_d[sl, :])
            nc.sync.dma_start(u1[k][:, :], u1_d[sl, :])
            nc.sync.dma_start(wf[k][:, :], wfc_d[sl, :])
        nc.sync.dma_start(bfct[:, :], bfct_d[:, :])
        if with_bias:
            nc.sync.dma_start(b0r[:, :], b0r_d[:, :])
            nc.sync.dma_start(b1r[:, :], b1r_d[:, :])
            nc.vector.memset(ones[:, :], 1.0)

        for p in range(2):
            nc.vector.memset(h0t[p][:, :], 0.0)
        nc.vector.memset(c0[:, :], 0.0)
        nc.vector.memset(c1[:, :], 0.0)
        nc.vector.memset(h1[:, :], 0.0)

        xall_r = xall[:, :].rearrange("p (b t) -> p b t", b=BL)

        def zx_r(z):
            """(if_view, og_view), each [p, m(4), b, t]."""
            return tuple(zz[:, :].rearrange("p (m b t) -> p m b t",
                                            m=NM // 2, b=BL) for zz in z)

        def _zx_tile(z, m):
            """(tile, local m) for global gate chunk m."""
            return (z[0], m) if m < 4 else (z[1], m - 4)

        def h0t_r(p):
            return h0t[p][:, :].rearrange("p (k b t) -> p k b t", k=KT, b=BL)

        def proj0(zx, toff):
            """xw0 = x[:, :, toff:toff+CHUNK] @ W0 -> PSUM zx."""
            if isinstance(toff, int):
                rhs = xall_r[:, :, toff:toff + CHUNK]
            else:
                rhs = xall_r[:, :, bass.ds(toff, CHUNK)]
            for m in range(NM):
                zt, ml = _zx_tile(zx, m)
                msl = slice(m * 128, (m + 1) * 128)
                nc.tensor.matmul(zt[:, ml * 128:(ml + 1) * 128],
                                 lhsT=w0[:, msl], rhs=rhs,
                                 start=(ml == 0), stop=False)
            if with_bias:
                for m in range(NM):
                    zt, ml = _zx_tile(zx, m)
                    msl = slice(m * 128, (m + 1) * 128)
                    nc.tensor.matmul(zt[:, ml * 128:(ml + 1) * 128],
                                     lhsT=b0r[:, msl],
                                     rhs=ones[:, :], start=False, stop=False)

        def proj1(zx, hp):
            """xw1 = h0t[hp] @ W1 -> PSUM zx."""
            for m in range(NM):
                zt, ml = _zx_tile(zx, m)
                msl = slice(m * 128, (m + 1) * 128)
                for k in range(KT):
                    nc.tensor.matmul(
                        zt[:, ml * 128:(ml + 1) * 128], lhsT=w1[k][:, msl],
                        rhs=h0t[hp][:, k * BL * CHUNK:(k + 1) * BL * CHUNK],
                        start=(ml == 0 and k == 0), stop=False)
            if with_bias:
                for m in range(NM):
                    zt, ml = _zx_tile(zx, m)
                    msl = slice(m * 128, (m + 1) * 128)
                    nc.tensor.matmul(zt[:, ml * 128:(ml + 1) * 128],
                                     lhsT=b1r[:, msl],
                                     rhs=ones[:, :], start=False, stop=False)

        def rec_mms(zxr, t, u, h_rhs, mid_emit=None):
            """z[:, :, t] += U^T @ h for one step.

            stop=True only on the last matmul per PSUM bank per chunk epoch
            (the group-started flag is bank-granular).  mid_emit (if given)
            is called after the i/f gate matmuls (m0-m3, the bank-0 tile) so
            the i/f sigmoid runs concurrently with the o/g matmuls."""
            zir, zgr = zxr
            for m in range(NM):
                if m == 4 and mid_emit is not None:
                    mid_emit()
                zr = zir if m < 4 else zgr
                ml = m % 4
                msl = slice(m * 128, (m + 1) * 128)
                for k in range(KT):
                    stop = (t == CHUNK - 1 and k == KT - 1 and ml == 3)
                    nc.tensor.matmul(zr[:, ml, :, t], lhsT=u[k][:, msl],
                                     rhs=h_rhs(k),
                                     start=False, stop=stop)

        def sig_emit(zxr, t, lname):
            """i/f sigmoid (bank-0 tile only) — emitted between the m3 and m4
            matmuls so it runs concurrently with the o/g matmuls and leaves
            the critical ring."""
            zir, _ = zxr
            sig = work.tile([128, 6 * BL], f32, tag=f"sig{lname}")
            sig_r = sig[:, :].rearrange("p (m b) -> p m b", m=6)
            nc.scalar.activation(sig_r[:, 0:4, :], zir[:, 0:4, :, t],
                                 AF.Sigmoid)
            return sig_r

        def gate_math(zxr, t, cst, h_out_ap, lname, sig_r):
            """o-sigmoid + cell update + h for one step; z/g read from PSUM.
            fc is emitted before ig: it only needs the early i/f sigmoid, so
            it runs during the o/g matmuls, off the critical ring."""
            _, zgr = zxr
            nc.scalar.activation(sig_r[:, 4:6, :], zgr[:, 0:2, :, t],
                                 AF.Sigmoid)
            ig = work.tile([128, KT * BL], f32, tag=f"ig{lname}")
            ig_r = ig[:, :].rearrange("p (k b) -> p k b", k=KT)
            cst_r = cst[:, :].rearrange("p (k b) -> p k b", k=KT)
            nc.vector.tensor_tensor(cst_r, sig_r[:, 2:4, :], cst_r, ALU.mult)
            nc.vector.tensor_tensor(ig_r, sig_r[:, 0:2, :],
                                    zgr[:, 2:4, :, t], ALU.mult)
            nc.vector.tensor_tensor(cst_r, cst_r, ig_r, ALU.add)
            nc.vector.tensor_tensor(h_out_ap, sig_r[:, 4:6, :], cst_r,
                                    ALU.mult)

        def l0_step(t, zp, hp):
            """Layer-0 step t of the chunk in h0t[hp] / zx0[zp]."""
            zr = zx_r(zx0[zp])
            hr = h0t_r(hp)
            hprev = h0t_r(1 - hp)

            def h_rhs(k):
                if t == 0:
                    return hprev[:, k, :, CHUNK - 1]
                return hr[:, k, :, t - 1]

            if DIAG_MODE == "mm_chain":
                rec_mms(zr, t, u0, h_rhs)
                # timing-only: 1-op DVE chain (PSUM->h copy) between bursts
                nc.vector.tensor_copy(hr[:, :, :, t], zr[0][:, 0:2, :, t])
            elif DIAG_MODE == "mm_only":
                rec_mms(zr, t, u0, h_rhs)
            elif DIAG_MODE == "vec_only":
                sig_r = sig_emit(zr, t, "0")
                gate_math(zr, t, c0, hr[:, :, :, t], "0", sig_r)
            else:
                sig_holder = []
                rec_mms(zr, t, u0, h_rhs,
                        mid_emit=lambda: sig_holder.append(
                            sig_emit(zr, t, "0")))
                gate_math(zr, t, c0, hr[:, :, :, t], "0", sig_holder[0])

        def l1_step(t, zp):
            zr = zx_r(zx1[zp])
            h1_r = h1[:, :].rearrange("p (k b) -> p k b", k=KT)

            def h_rhs(k):
                return h1_r[:, k, :]

            if DIAG_MODE == "mm_chain":
                rec_mms(zr, t, u1, h_rhs)
                nc.vector.tensor_copy(h1_r, zr[0][:, 0:2, :, t])
            elif DIAG_MODE == "mm_only":
                rec_mms(zr, t, u1, h_rhs)
            elif DIAG_MODE == "vec_only":
                sig_r = sig_emit(zr, t, "1")
                gate_math(zr, t, c1, h1_r, "1", sig_r)
            else:
                sig_holder = []
                rec_mms(zr, t, u1, h_rhs,
                        mid_emit=lambda: sig_holder.append(
                            sig_emit(zr, t, "1")))
                gate_math(zr, t, c1, h1_r, "1", sig_holder[0])

        def half(l1_par, l0_par, l0_off, skip_l0=False, skip_l1=False):
            """One pipeline half: L0 on chunk with buffers l0_par at x-offset
            l0_off, interleaved with L1 on the chunk held in parity l1_par."""
            if not skip_l0:
                proj0(zx0[l0_par], l0_off)
            if not skip_l1:
                proj1(zx1[l1_par], l1_par)
            for t in range(CHUNK):
                if not skip_l0:
                    l0_step(t, l0_par, l0_par)
                if not skip_l1:
                    l1_step(t, l1_par)

        # ---- prologue: L0 chunk 0 (parity 0) ----
        half(0, 0, 0, skip_l1=True)

        nch = NCH if sim_nch is None else sim_nch
        if sim_nch is not None:
            # fully unrolled small-T variant for TimelineSim (no registers)
            for i in range(nch // 2 - 1):
                half(0, 1, (2 * i + 1) * CHUNK)
                half(1, 0, (2 * i + 2) * CHUNK)
        else:
            # steady state: body i does L1(2i),L0(2i+1) ; L1(2i+1),L0(2i+2)
            rep_ctx = tc.For_i(0, nrep) if nrep > 1 else None
            if rep_ctx is not None:
                rep_ctx.__enter__()
            with tc.For_i(0, (NCH // 2 - 1) * 2 * CHUNK, 2 * CHUNK) as iv:
                half(0, 1, iv + CHUNK)
                half(1, 0, iv + 2 * CHUNK)
            if rep_ctx is not None:
                rep_ctx.__exit__(None, None, None)

        # ---- epilogue: L1(nch-2) + L0(nch-1); then L1(nch-1) ----
        half(0, 1, (nch - 1) * CHUNK)
        half(1, 0, 0, skip_l0=True)

        # ---- FC head: out^T = Wfc^T @ h1^T + bfc ----
        psf = zx1[0][1][:, 0:BL]
        for k in range(KT):
            nc.tensor.matmul(psf, lhsT=wf[k][:, :],
                             rhs=h1[:, k * BL:(k + 1) * BL],
                             start=(k == 0), stop=(k == KT - 1))
        oT = work.tile([128, BL], f32, tag="oT")
        nc.scalar.activation(oT[:, :], psf, AF.Identity, bias=bfct[:, 0:1])
        nc.sync.dma_start(out_d[:, :], oT[:, :])

    nc.compile()
    return nc


def _get_compiled(with_bias=False):
    key = ("bias" if with_bias else "main")
    if key not in _cache:
        _cache[key] = _build(with_bias=with_bias)
    return _cache[key]


def _in_maps(input_seq, W0, U0, b0, W1, U1, b1, Wfc, bfc, with_bias=False):
    import ml_dtypes
    mdt = ml_dtypes.bfloat16
    x = np.asarray(input_seq, dtype=np.float32)
    # reorder gate blocks (i,f,g,o) -> (i,f,o,g)
    perm = np.concatenate([np.arange(0, 2 * H),
                           np.arange(3 * H, 4 * H),
                           np.arange(2 * H, 3 * H)])

    def gp(w):
        return np.ascontiguousarray(
            np.asarray(w, np.float32)[..., perm].astype(mdt))

    shared = {
        "w0": gp(W0),
        "u0": gp(U0),
        "w1": gp(W1),
        "u1": gp(U1),
        "wfc": np.ascontiguousarray(np.asarray(Wfc, np.float32).astype(mdt)),
        "bfct": np.ascontiguousarray(
            np.asarray(bfc, np.float32).reshape(1, 128).T),
    }
    if with_bias:
        shared["b0r"] = gp(np.asarray(b0, np.float32).reshape(1, G4))
        shared["b1r"] = gp(np.asarray(b1, np.float32).reshape(1, G4))
    in_maps = []
    for c in range(NCORES):
        xs = x[c * BL:(c + 1) * BL]                       # [BL, T, I]
        xp = np.ascontiguousarray(xs.transpose(2, 0, 1).astype(mdt))
        m = dict(shared)
        m["xprep"] = xp
        in_maps.append(m)
    return in_maps


def _run(nc, inputs, with_bias=False):
    from concourse.bass_utils import run_bass_kernel_spmd
    in_maps = _in_maps(**inputs, with_bias=with_bias)
    res = run_bass_kernel_spmd(nc, in_maps, list(range(NCORES)))
    out = np.empty((B, 1, O), np.float32)
    for c in range(NCORES):
        out[c * BL:(c + 1) * BL, 0, :] = res.results[c]["outT"].T
    return out


def kernel(input_seq, W0, U0, b0, W1, U1, b1, Wfc, bfc):
    with_bias = bool(np.any(np.asarray(b0)) or np.any(np.asarray(b1)))
    nc = _get_compiled(with_bias=with_bias)
    return _run(nc, dict(input_seq=input_seq, W0=W0, U0=U0, b0=b0, W1=W1,
                         U1=U1, b1=b1, Wfc=Wfc, bfc=bfc), with_bias=with_bias)
